# revision 11
# baseline (speedup 1.0000x reference)
"""Distributed Trainium2 Bass kernel for the AttGNN (2x GATConv branches +
global mean pool + fc) problem.

Strategy (8 NeuronCores):
  - Branch parallel: cores 0-3 run branch (x, adj), cores 4-7 run
    (wt_x, wt_adj) concurrently (weights shared, SPMD graph identical).
  - Node phase is REPLICATED: every core computes h = x@W for all 20480
    slots (160 windows) and writes the packed node table to its own DRAM.
    This removes the 4-core AllGather of the node table entirely (the ring
    collective costs ~90us per chunk in latency + ~62 GB/s bus, far more
    than the extra 3x matmul work on the otherwise-idle PE).
  - Within a 4-core group: destination-node sharding for the edge phase.
    Nodes are remapped on the host into windows of 128 slots (127 real +
    1 pad), LPT-balanced by in-degree. Each core's own 40 dst windows are
    numbered first in its (per-core, host-permuted) table layout so the
    SPMD program is identical across cores.
  - Packed row (768B, gather granularity must be a multiple of 256B):
    [h0*128 fp8 | 1.0 | h1*128 | 1.0 | h2*128 | 1.0 | h3*128 | 1.0 |
     a_src bf16 x4 | pad]. The ones columns make each scatter matmul
    produce the softmax denominator in PSUM column 128 for free.
  - Edge phase per window: one dma_gather fetches packed rows by edge src
    (768B each, 4 SWDGE queues round-robin). a_dst is expanded to edges
    with a matmul against a host-built fp8 one-hot (stp). The exp(logit)
    per (edge, head) is applied to the scatter one-hot (sfp) as 68 small
    tensor_scalar "scaled copies" (fp8 runs at 2 elem/cycle in 2x_2p mode)
    spread across DVE, ACT, and GPSIMD, instead of multiplying the 512-wide
    h payload on DVE (which is fp8-bound to 1 elem/cycle). The scatter
    matmuls then consume the gathered fp8 h directly.
  - Masked column-sum matmul accumulates the global mean pool; a tiny
    8-core AllGather + on-device fc1 finishes both branches redundantly.

Host-side work is restricted to topology preprocessing (edge partitioning,
index/layout construction) and weight folding plus dtype casts; all FLOPs
on float data happen on device.
"""

import sys

if "/opt/trn_rl_repo" not in sys.path:
    sys.path.insert(0, "/opt/trn_rl_repo")

import heapq

import numpy as np
import ml_dtypes

BF16 = ml_dtypes.bfloat16
FP8 = ml_dtypes.float8_e4m3

N = 20000
F = 512
HEADS = 4
HC = 512
NCORES = 8
GROUP = 4
NW = 40                 # edge windows per core
NWG = GROUP * NW        # node windows globally (160)
WSLOT = 128             # slots per window (127 real + 1 pad)
CAP = 127
GSLOTS = NWG * WSLOT    # 20480 slots in the (replicated) node table
ROWB = 768              # packed row bytes (gather elem must be %256)
NEG_ATT = 0.2
NEG_ACT = 0.01

# engine split for the 68 scaled-one-hot copies per window:
# index m = (t*4+h) % 17 -> ACT for m in {1,5,9}, Pool for m in {3,11,13},
# DVE otherwise (44 DVE / 12 ACT / 12 Pool per window of tu=17).
_TS_ACT = (1, 5, 9)
_TS_POOL = (3, 11, 13)
NB = 4                  # node windows packed per DMA transfer


# --------------------------------------------------------------------------
# host-side topology preprocessing
# --------------------------------------------------------------------------

def _lpt_assign(indeg):
    """Assign nodes to NWG bins balanced by in-degree, <=CAP nodes/bin."""
    nbins = NWG
    order = np.argsort(-indeg, kind="stable")
    load = np.zeros(nbins, dtype=np.int64)
    nnodes = np.zeros(nbins, dtype=np.int32)
    heap = [(0, b) for b in range(nbins)]
    heapq.heapify(heap)
    win_of = np.empty(N, np.int32)
    pos_of = np.empty(N, np.int32)
    for n in order:
        while True:
            l, b = heapq.heappop(heap)
            if nnodes[b] >= CAP or l != load[b]:
                continue
            break
        win_of[n] = b
        pos_of[n] = nnodes[b]
        nnodes[b] += 1
        load[b] += int(indeg[n])
        if nnodes[b] < CAP:
            heapq.heappush(heap, (int(load[b]), b))
    return win_of, pos_of, nnodes


def _prep_branch(x, adj):
    src0 = np.asarray(adj[0], dtype=np.int64)
    dst0 = np.asarray(adj[1], dtype=np.int64)
    loops = np.arange(N, dtype=np.int64)
    src = np.concatenate([src0, loops])
    dst = np.concatenate([dst0, loops])
    indeg = np.bincount(dst, minlength=N)
    win_of, pos_of, nnodes = _lpt_assign(indeg)
    cnt = np.bincount(win_of[dst], minlength=NWG)
    tu = int(np.ceil(cnt.max() / 128.0))
    return dict(
        x=np.asarray(x, dtype=np.float32),
        src=src, dst=dst,
        win_of=win_of, pos_of=pos_of, nnodes=nnodes, tu=tu,
    )


def _wrap_idx(a):
    """[M] -> dma_gather idx layout [128, M//16] (wrapped, replicated x8)."""
    return np.ascontiguousarray(np.tile(a.reshape(-1, 16).T, (8, 1)).astype(np.int16))


def _finalize_branch(pb, tu):
    ew = tu * 128
    win_of, pos_of = pb["win_of"], pb["pos_of"]
    src, dst = pb["src"], pb["dst"]
    per_core = []
    for c in range(GROUP):
        # local window permutation: own 40 dst windows first, rest after
        own = np.arange(c * NW, (c + 1) * NW)
        rest = np.concatenate([np.arange(0, c * NW), np.arange((c + 1) * NW, NWG)])
        g2l = np.empty(NWG, np.int64)
        g2l[own] = np.arange(NW)
        g2l[rest] = np.arange(NW, NWG)
        lwin_of = g2l[win_of]                      # local window per node
        slot_l = lwin_of * WSLOT + pos_of          # local table slot per node

        # node-phase input: [160, 128(feat%128), 4(k), 128(slotpos)] bf16
        xs = np.zeros((GSLOTS, F), np.float32)
        xs[slot_l] = pb["x"]
        xtw = np.ascontiguousarray(
            xs.reshape(NWG, WSLOT, 4, 128).transpose(0, 3, 2, 1)
            .reshape(NWG, 128, 512).astype(BF16))

        # this core's edges: dst in own windows (local windows 0..NW-1)
        emask = lwin_of[dst] < NW
        es_ = src[emask]
        ed_ = dst[emask]
        ebin = lwin_of[ed_]                        # 0..NW-1
        order = np.argsort(ebin, kind="stable")
        sb = ebin[order]
        ssrc = slot_l[es_[order]]
        sslot = pos_of[ed_[order]]
        counts = np.bincount(sb, minlength=NW)
        starts = np.zeros(NW, np.int64)
        np.cumsum(counts[:-1], out=starts[1:])
        within = np.arange(len(sb)) - starts[sb]
        srcpad = np.zeros((NW, ew), np.int64)
        slotpad = np.full((NW, ew), CAP, np.int64)
        srcpad[sb, within] = ssrc
        slotpad[sb, within] = sslot

        sidx = _wrap_idx(srcpad.reshape(-1))
        sv = slotpad.reshape(NW, tu, 128)
        onehot = sv[:, :, :, None] == np.arange(128)[None, None, None, :]
        # spp[w*128+r, 0, t*128+slot]: scatter one-hot (edges on partitions)
        # spp[w*128+r, 1, t*128+e]:    expand one-hot (slots on partitions)
        spp = np.empty((NW * 128, 2, tu * 128), FP8)
        spp[:, 0, :] = onehot.transpose(0, 2, 1, 3).reshape(NW * 128, tu * 128)
        spp[:, 1, :] = onehot.transpose(0, 3, 1, 2).reshape(NW * 128, tu * 128)
        cmask = np.zeros((128, NW), np.float32)
        for w in range(NW):
            cmask[: pb["nnodes"][c * NW + w], w] = 1.0
        per_core.append(dict(xtw=xtw, sidx=sidx, spp=np.ascontiguousarray(spp),
                             cmk=cmask.astype(BF16)))
    return per_core


# --------------------------------------------------------------------------
# device graph
# --------------------------------------------------------------------------

_BUILD_CACHE = {}


def _build(tu, use_bias):
    key = (tu, use_bias)
    if key in _BUILD_CACHE:
        return _BUILD_CACHE[key]

    from contextlib import ExitStack

    import concourse.bass as bass
    import concourse.mybir as mybir
    import concourse.bacc as bacc
    from concourse import tile
    from concourse.alu_op_type import AluOpType as AO

    f32 = mybir.dt.float32
    bf16 = mybir.dt.bfloat16
    i16 = mybir.dt.int16
    fp8 = mybir.dt.float8e4
    AF = mybir.ActivationFunctionType

    ew = tu * 128

    nc = bacc.Bacc("TRN2", target_bir_lowering=False, debug=False,
                   num_swdge_queues=4)
    xtw = nc.declare_dram_parameter("xtw", [NWG, 128, 512], bf16, isOutput=False)
    wm = nc.declare_dram_parameter("wm", [F, 520], f32, isOutput=False)
    f1t = nc.declare_dram_parameter("f1t", [F, F], f32, isOutput=False)
    f1b = nc.declare_dram_parameter("f1b", [128, 4], f32, isOutput=False)
    sidx = nc.declare_dram_parameter("sidx", [128, NW * tu * 8], i16, isOutput=False)
    spp = nc.declare_dram_parameter("spp", [NW * 128, 2, tu * 128], fp8, isOutput=False)
    cmk = nc.declare_dram_parameter("cmk", [128, NW], bf16, isOutput=False)
    if use_bias:
        brp = nc.declare_dram_parameter("brp", [128, 512], f32, isOutput=False)
    outp = nc.declare_dram_parameter("out", [128, 12], f32, isOutput=True)

    with tile.TileContext(nc) as tc, ExitStack() as ctx:
        dram = ctx.enter_context(tc.tile_pool(name="dram", bufs=1, space="DRAM"))
        htab = dram.tile([GSLOTS, ROWB], fp8)
        gsl = dram.tile([1, 512], f32)
        gsa = dram.tile([8, 512], f32, addr_space="Shared")

        const = ctx.enter_context(tc.tile_pool(name="const", bufs=1))
        wkb8 = const.tile([128, 4, 528], fp8)
        wab = const.tile([128, 4, 8], bf16)
        f1tb = const.tile([128, 4, 512], bf16)
        f1bs = const.tile([128, 4], f32)
        cmks = const.tile([128, NW], bf16)
        sidxs = const.tile([128, NW * tu * 8], i16)
        adl = const.tile([128, NWG, 4], bf16)
        ones = const.tile([128, 4, 1], fp8)
        al_att = const.tile([128, 1], f32)
        al_act = const.tile([128, 1], f32)
        nc.any.memset(al_att[:, :], NEG_ATT)
        nc.any.memset(al_act[:, :], NEG_ACT)
        nc.any.memset(ones[:, :, :], 1.0)
        nc.any.memset(wkb8[:, :, 512:528], 0.0)
        if use_bias:
            brps = const.tile([128, 512], f32)
            nc.sync.dma_start(brps[:, :], brp[:, :])

        with tc.tile_pool(name="stage", bufs=2) as stage:
            ws = stage.tile([128, 4, 520], f32, tag="st")
            for k in range(4):
                nc.sync.dma_start(ws[:, k, :], wm[k * 128:(k + 1) * 128, :])
            nc.vector.tensor_copy(wkb8[:, :, 0:512], ws[:, :, 0:512])
            nc.vector.tensor_copy(wab[:, :, :], ws[:, :, 512:520])
            fs = stage.tile([128, 4, 512], f32, tag="st")
            for k in range(4):
                nc.sync.dma_start(fs[:, k, :], f1t[k * 128:(k + 1) * 128, :])
            nc.vector.tensor_copy(f1tb[:, :, :], fs[:, :, :])
        nc.sync.dma_start(f1bs[:, :], f1b[:, :])
        nc.sync.dma_start(cmks[:, :], cmk[:, :])
        nc.sync.dma_start(sidxs[:, :], sidx[:, :])

        DR = mybir.MatmulPerfMode.DoubleRow

        # ---- node phase (replicated): h = x @ W, a = x @ [Wa_src|Wa_dst] ----
        with tc.tile_pool(name="xp", bufs=2) as xpool, \
             tc.tile_pool(name="hp", bufs=2) as hpool, \
             tc.tile_pool(name="p1ps", bufs=2, space="PSUM") as p1ps:
            for blk in range(NWG // NB):
                ch0 = blk * NB
                xb = xpool.tile([128, NB, 4, 128], bf16, tag="xb")
                nc.sync.dma_start(
                    xb[:, :, :, :],
                    xtw[ch0:ch0 + NB].rearrange("a p (k c) -> p a k c", c=128))
                xb8 = xpool.tile([128, NB, 4, 128], fp8, tag="xb8")
                nc.vector.tensor_copy(xb8[:, :, :, :], xb[:, :, :, :])
                hp = hpool.tile([128, NB, ROWB], fp8, tag="hp")
                for a in range(NB):
                    ch = ch0 + a
                    ph = p1ps.tile([128, 512], f32, tag="ph")
                    pa = p1ps.tile([128, 8], f32, tag="pa")
                    for j in range(2):
                        nc.tensor.matmul(ph[:, :], xb8[:, a, 2 * j:2 * j + 2, :],
                                         wkb8[:, 2 * j:2 * j + 2, 0:512],
                                         start=(j == 0), stop=(j == 1),
                                         perf_mode=DR)
                    for k in range(4):
                        nc.tensor.matmul(pa[:, :], xb[:, a, k, :], wab[:, k, :],
                                         start=(k == 0), stop=(k == 3))
                    hv = hp[:, a, 0:516].rearrange("p (b x) -> p b x", x=129)
                    nc.scalar.activation(
                        hv[:, :, 0:128],
                        ph[:, :].rearrange("p (b x) -> p b x", x=128), AF.Copy)
                    nc.gpsimd.tensor_copy(hv[:, :, 128:129], ones[:, :, :])
                    nc.vector.tensor_copy(
                        hp[:, a, 516:524].bitcast(bf16), pa[:, 0:4])
                    nc.vector.tensor_copy(adl[:, ch, :], pa[:, 4:8])
                nc.sync.dma_start(
                    htab[ch0 * 128:(ch0 + NB) * 128, 0:524].rearrange(
                        "(a p) r -> p a r", p=128),
                    hp[:, :, 0:524])

        # ---- edge phase (software-pipelined: loads 2 ahead, logits 1 ahead,
        # finish ops 1 behind so no engine queue ever waits on the PE) ----
        with tc.tile_pool(name="gp", bufs=3) as gpool, \
             tc.tile_pool(name="sp", bufs=3) as spool, \
             tc.tile_pool(name="vp", bufs=2) as vpool, \
             tc.tile_pool(name="ap2", bufs=3) as apool, \
             tc.tile_pool(name="p3ps", bufs=2, space="PSUM") as p3ps, \
             tc.tile_pool(name="pcps", bufs=1, space="PSUM") as pcps:
            pc_ = pcps.tile([1, 512], f32, tag="pC")
            live = {}

            def stage_a1(w):
                """Issue the gather + one-hot load."""
                hpt = gpool.tile([128, tu, ROWB], fp8, tag="hpt", bufs=4)
                nc.gpsimd.dma_gather(
                    hpt[:, :, :], htab[:, :],
                    sidxs[:, w * tu * 8:(w + 1) * tu * 8], ew, ew, ROWB,
                    single_packet=False, queue_num=w % 4)
                sgt = spool.tile([128, 2, tu, 128], fp8, tag="sgt", bufs=4)
                nc.sync.dma_start(
                    sgt[:, :, :, :],
                    spp[w * 128:(w + 1) * 128, :, :].rearrange(
                        "p a (t c) -> p a t c", c=128))
                live[w] = [hpt, sgt]

            def stage_a2(w):
                """a_dst expand + logits + exp (inputs arrived long ago)."""
                hpt, sgt = live[w]
                pE = p3ps.tile([128, tu, 4], f32, tag="pE")
                for t in range(tu):
                    nc.tensor.matmul(pE[:, t, :], sgt[:, 1, t, :],
                                     adl[:, w, :], start=True, stop=True,
                                     skip_group_check=True)
                es = apool.tile([128, tu, 4], f32, tag="es", bufs=2)
                nc.vector.tensor_tensor(
                    es[:, :, :], hpt[:, :, 516:524].bitcast(bf16),
                    pE[:, :, :], AO.add)
                el = apool.tile([128, tu, 4], f32, tag="el", bufs=2)
                nc.scalar.activation(el[:, :, :], es[:, :, :], AF.Prelu,
                                     alpha=al_att[:, :])
                ex = apool.tile([128, tu, 4], f32, tag="ex", bufs=3)
                nc.scalar.activation(ex[:, :, :], el[:, :, :], AF.Exp)
                live[w].append(ex)

            def stage_b1(w):
                """Scaled one-hots (DVE/ACT/Pool) + scatter matmuls (PE)."""
                hpt, sgt, ex = live[w]
                V = vpool.tile([128, tu, 4, 128], fp8, tag="V")
                for t in range(tu):
                    for h in range(HEADS):
                        m = (t * 4 + h) % 17
                        if m in _TS_ACT:
                            nc.scalar.activation(
                                V[:, t, h, :], sgt[:, 0, t, :], AF.Copy,
                                scale=ex[:, t, h:h + 1])
                        elif m in _TS_POOL:
                            nc.gpsimd.tensor_scalar(
                                V[:, t, h, :], sgt[:, 0, t, :],
                                ex[:, t, h:h + 1], None, AO.mult)
                        else:
                            nc.vector.tensor_scalar(
                                V[:, t, h, :], sgt[:, 0, t, :],
                                ex[:, t, h:h + 1], None, AO.mult)

                pOa = p3ps.tile([128, 2, 129], f32, tag="pOa")
                pOb = p3ps.tile([128, 2, 129], f32, tag="pOb")
                for t in range(tu):
                    for h in range(HEADS):
                        po = pOa if h < 2 else pOb
                        nc.tensor.matmul(
                            po[:, h % 2, :], V[:, t, h, :],
                            hpt[:, t, h * 129:(h + 1) * 129],
                            start=(t == 0), stop=(t == tu - 1),
                            skip_group_check=True)
                live[w] = (pOa, pOb)

            def stage_b2(w):
                """Normalize + activation + pool accumulate."""
                pOa, pOb = live.pop(w)
                # denominators live in column 128 of each head's PSUM slice
                rd = apool.tile([128, 4], f32, tag="rd", bufs=2)
                nc.vector.tensor_copy(
                    rd[:, 0:2], pOa[:, :, 128:129].rearrange("p a b -> p (a b)"))
                nc.vector.tensor_copy(
                    rd[:, 2:4], pOb[:, :, 128:129].rearrange("p a b -> p (a b)"))
                de = apool.tile([128, 4], f32, tag="de", bufs=2)
                nc.vector.tensor_scalar_add(de[:, :], rd[:, :], 1e-16)
                rc = apool.tile([128, 4], f32, tag="rc", bufs=2)
                nc.vector.reciprocal(rc[:, :], de[:, :])

                ab = spool.tile([128, 4, 128], bf16, tag="ab")
                if use_bias:
                    nb = spool.tile([128, 4, 128], f32, tag="nb")
                    for h in range(4):
                        po = pOa if h < 2 else pOb
                        nc.vector.tensor_scalar(
                            nb[:, h, :], po[:, h % 2, 0:128],
                            rc[:, h:h + 1], None, AO.mult)
                    nc.vector.tensor_tensor(
                        nb[:, :, :], nb[:, :, :],
                        brps[:, :].rearrange("p (b x) -> p b x", x=128), AO.add)
                    nc.scalar.activation(ab[:, :, :], nb[:, :, :], AF.Prelu,
                                         alpha=al_act[:, :])
                else:
                    for h in range(4):
                        po = pOa if h < 2 else pOb
                        nc.scalar.activation(
                            ab[:, h, :], po[:, h % 2, 0:128],
                            AF.Prelu, scale=rc[:, h:h + 1], alpha=al_act[:, :])

                nc.tensor.matmul(pc_[:, :], cmks[:, w:w + 1],
                                 ab[:, :, :].rearrange("p a b -> p (a b)"),
                                 start=(w == 0), stop=(w == NW - 1),
                                 skip_group_check=True)

            stage_a1(0)
            stage_a1(1)
            stage_a1(2)
            stage_a2(0)
            stage_a2(1)
            stage_b1(0)
            for w in range(NW):
                if w + 3 < NW:
                    stage_a1(w + 3)
                if w + 1 < NW:
                    stage_b1(w + 1)
                if w + 2 < NW:
                    stage_a2(w + 2)
                stage_b2(w)

            # ---- global mean + fc1 (redundant on every core) ----
            with tc.tile_pool(name="p4", bufs=1) as p4:
                gs = p4.tile([1, 512], f32)
                nc.vector.tensor_scalar(gs[:, :], pc_[:, :], 1.0 / N, None, AO.mult)
                nc.sync.dma_start(gsl[:, :], gs[:, :])
                nc.gpsimd.collective_compute(
                    "AllGather", AO.bypass,
                    replica_groups=[[0, 1, 2, 3, 4, 5, 6, 7]],
                    ins=[gsl[:, :]], outs=[gsa[:, :]])
                gtr = p4.tile([128, 4, 8], f32)
                for r in range(8):
                    nc.sync.dma_start(
                        gtr[:, :, r],
                        gsa[r:r + 1, :].rearrange("o (c p) -> (o p) c", p=128))
                gt = p4.tile([128, 4, 2], f32)
                nc.vector.reduce_sum(
                    gt[:, :, :],
                    gtr[:, :, :].rearrange("p c (g k) -> p c g k", k=4),
                    mybir.AxisListType.X)
                gtb = p4.tile([128, 4, 2], bf16)
                nc.vector.tensor_copy(gtb[:, :, :], gt[:, :, :])
                pF = pcps.tile([128, 8], f32, tag="pF")
                for m in range(4):
                    for k in range(4):
                        nc.tensor.matmul(
                            pF[:, m * 2:(m + 1) * 2],
                            f1tb[:, k, m * 128:(m + 1) * 128], gtb[:, k, :],
                            start=(k == 0), stop=(k == 3),
                            skip_group_check=True)
                fo = p4.tile([128, 4, 3], f32)
                for m in range(4):
                    nc.scalar.activation(fo[:, m, 0:2], pF[:, m * 2:(m + 1) * 2],
                                         AF.Prelu, bias=f1bs[:, m:m + 1],
                                         alpha=al_act[:, :])
                nc.vector.tensor_tensor(fo[:, :, 2:3], fo[:, :, 0:1],
                                        fo[:, :, 1:2], AO.subtract)
                nc.sync.dma_start(outp[:, :], fo[:, :, :])

    nc.compile()
    _BUILD_CACHE[key] = nc
    return nc


# --------------------------------------------------------------------------
# entry point
# --------------------------------------------------------------------------

def kernel(x, adj, wt_x, wt_adj, W, att_src, att_dst, bias, fc1_w, fc1_b):
    from concourse.bass_utils import run_bass_kernel_spmd

    x = np.asarray(x, np.float32)
    wt_x = np.asarray(wt_x, np.float32)
    adj = np.asarray(adj)
    wt_adj = np.asarray(wt_adj)
    W = np.asarray(W, np.float32)
    att_src = np.asarray(att_src, np.float32)
    att_dst = np.asarray(att_dst, np.float32)
    bias = np.asarray(bias, np.float32)
    fc1_w = np.asarray(fc1_w, np.float32)
    fc1_b = np.asarray(fc1_b, np.float32)

    pba = _prep_branch(x, adj)
    pbb = _prep_branch(wt_x, wt_adj)
    tu = max(pba["tu"], pbb["tu"])
    pca = _finalize_branch(pba, tu)
    pcb = _finalize_branch(pbb, tu)

    wa_s = np.einsum("fhc,hc->fh", W.reshape(F, HEADS, 128), att_src)
    wa_d = np.einsum("fhc,hc->fh", W.reshape(F, HEADS, 128), att_dst)
    wmat = np.ascontiguousarray(
        np.concatenate([W, wa_s, wa_d], axis=1).astype(np.float32))
    f1br = np.ascontiguousarray(fc1_b.reshape(4, 128).T.astype(np.float32))
    use_bias = bool(np.any(bias != 0.0))

    nc = _build(tu, use_bias)

    in_maps = []
    for core in range(NCORES):
        pc = (pca if core < GROUP else pcb)[core % GROUP]
        m = dict(xtw=pc["xtw"], wm=wmat, f1t=fc1_w, f1b=f1br,
                 sidx=pc["sidx"], spp=pc["spp"], cmk=pc["cmk"])
        if use_bias:
            m["brp"] = np.ascontiguousarray(
                np.tile(bias[None, :], (128, 1)).astype(np.float32))
        in_maps.append(m)

    kernel.last_nc = nc
    kernel.last_in_maps = in_maps

    trace = bool(int(__import__("os").environ.get("GNN_TRACE", "0")))
    res = None
    for attempt in range(4):
        try:
            res = run_bass_kernel_spmd(nc, in_maps,
                                       core_ids=list(range(NCORES)),
                                       trace=trace)
            break
        except Exception:
            # axon runs occasionally hit transient device errors; retry
            if attempt == 3:
                raise
            import time as _time
            _time.sleep(3)
    kernel.last_exec_time_ns = res.exec_time_ns
    kernel.last_res = res
    o = np.asarray(res.results[0]["out"]).reshape(128, 4, 3)
    o2 = o.transpose(1, 0, 2).reshape(512, 3)
    return np.ascontiguousarray(
        np.concatenate([o2[:, 0], o2[:, 1], o2[:, 2]])[None, :]).astype(np.float32)
